# revision 23
# baseline (speedup 1.0000x reference)
"""MultiHeadedAttention Trainium2 kernel.

Problem: B=2, S=4096, d_model=512, H=8 heads, dk=64.
  q/k/v proj -> scaled dot-product attention per head -> concat -> out proj.

Sharding: 8 cores = (batch b in {0,1}) x (head-pair hp in {0..3}).
Each core computes, for its batch and its 2 heads:
  QpT/KpT = (x @ W[:, hp*128:hp*128+128] + b).T          [128=2*dk, 4096]
  VpT     = same, stored bf16; transposed to s-major vp via DMA xbar
            transpose (with a softmax-denominator ones column appended).
  Per q-tile of 512, per head (head-sequential passes):
    S^T[k, q]  = Kp_h Qp_h^T   (fp32r matmuls, k-major, 3 k-blocks per
                                PSUM chunk of [128, 1536])
    P^T        = exp(S^T / 8)  (one ScalarE exp per chunk -> bf16)
    ctxQ, Z    = P^T-block^T @ [Vp_h | 1]  (bf16 matmuls, q-major out
                 [128q, 65] so all ctx matmuls of a pass share one PSUM
                 bank; the ones column gives the softmax denominator)
    ctxQ      /= Z  (reciprocal + per-partition scalar multiplies)
  ctxT = transpose(ctxQ);  outT_partial = Wo^T-chunks @ ctxT
Host: out[b] = sum_hp(outT_partial).T + bo.

Schedule: ScalarE's exp is the roofline (~251us busy).  The ctx stream
lags the scores/exp stream by CTX_LAG chunks globally (across pass and
q-tile boundaries) so the in-order PE queue never blocks on a fresh
ACT semaphore.  Projections (DMA kick and matmuls as separate steps),
normalize, transposes and the output projection are spread across
chunk boundaries as jobs.  PSUM: scores ring 2x3 banks + ctx
accumulator 1 bank + matmul scratch 1 bank = 8 banks.
"""

import numpy as np

import concourse.bass as bass
import concourse.bacc as bacc
import concourse.mybir as mybir
import concourse.tile as tile
from concourse.bass_utils import run_bass_kernel_spmd
from concourse.masks import make_identity

F32 = mybir.dt.float32
F32R = mybir.dt.float32r
BF16 = mybir.dt.bfloat16
EXP = mybir.ActivationFunctionType.Exp

B = 2
S = 4096
D = 512           # d_model
H = 8
DK = 64
HP = 4            # head pairs per batch
DL = 128          # local channels per core (2 heads)
CJ = 4            # contraction chunks of 128 over d_model
QT = S // 512     # 8 q-tiles of 512
KB = S // 128     # 32 k-blocks of 128
SCALE = 1.0 / np.sqrt(DK).item()  # 1/8
CHUNKS = [(3 * c, min(3 * c + 3, KB)) for c in range(11)]  # k-blocks per chunk
CTX_LAG = 5  # chunks the ctx stream lags the scores/exp stream by

TRACE = False
LAST_RESULTS = None

_prog_cache = {}


def _emit(nc, reps=1):
    assert reps == 1
    xqT = nc.declare_dram_parameter("xqT", [D, S], BF16, isOutput=False)
    xkT = nc.declare_dram_parameter("xkT", [D, S], BF16, isOutput=False)
    xvT = nc.declare_dram_parameter("xvT", [D, S], BF16, isOutput=False)
    wq = nc.declare_dram_parameter("wq", [D, DL], BF16, isOutput=False)
    wk = nc.declare_dram_parameter("wk", [D, DL], BF16, isOutput=False)
    wv = nc.declare_dram_parameter("wv", [D, DL], BF16, isOutput=False)
    bq = nc.declare_dram_parameter("bq", [DL, 1], F32, isOutput=False)
    bk = nc.declare_dram_parameter("bk", [DL, 1], F32, isOutput=False)
    bv = nc.declare_dram_parameter("bv", [DL, 1], F32, isOutput=False)
    wo = nc.declare_dram_parameter("wo", [DL, D], BF16, isOutput=False)
    outT = nc.declare_dram_parameter("outT", [D, S], BF16, isOutput=True)

    with tile.TileContext(nc) as tc:
        with (
            nc.allow_low_precision(reason="fp32r/bf16 matmul inputs"),
            tc.tile_pool(name="singles", bufs=1) as singles,
            tc.tile_pool(name="xin", bufs=14) as xin,
            tc.tile_pool(name="proj", bufs=1) as proj,
            tc.tile_pool(name="pt", bufs=8) as ptpool,
            tc.tile_pool(name="ctx", bufs=2) as ctxpool,
            tc.tile_pool(name="outp", bufs=4) as outpool,
            tc.tile_pool(name="norm", bufs=2) as normpool,
            tc.tile_pool(name="spring", bufs=2, space="PSUM") as spring,
            tc.tile_pool(name="cpsp", bufs=1, space="PSUM") as cpsp,
            tc.tile_pool(name="mm", bufs=1, space="PSUM") as mmps,
        ):
            # --- constants / weights (phase 0) ---
            warm_src = singles.tile([1, 8], F32, tag="warm_src")
            nc.vector.memset(warm_src, 1.0)
            warm = singles.tile([1, 8], F32, tag="warm")
            nc.scalar.activation(warm, warm_src, EXP, scale=1.0)

            ident = singles.tile([128, 128], BF16, tag="ident")
            make_identity(nc, ident)
            zeros260 = singles.tile([128, 260], BF16, tag="zeros260")
            nc.vector.memset(zeros260, 0.0)

            w_sb = {}
            b_sb = {}

            def load_w(name, w, bias):
                t = singles.tile([128, CJ, DL], BF16, tag=name)
                nc.sync.dma_start(
                    out=t, in_=w[:].rearrange("(j p) d -> p j d", p=128)
                )
                w_sb[name] = t
                bt = singles.tile([DL, 1], F32, tag="b" + name[1])
                nc.sync.dma_start(out=bt, in_=bias[:])
                b_sb["b" + name[1]] = bt

            wo_bf = singles.tile([DL, D], BF16, tag="wo_bf")

            def wo_job():
                nc.sync.dma_start(out=wo_bf, in_=wo[:])

            qpT = proj.tile([DL, S], BF16, tag="qpT")
            kpT = proj.tile([DL, S], BF16, tag="kpT")
            vpT = proj.tile([DL, S], BF16, tag="vpT")
            vp = proj.tile([128, KB, 130], BF16, tag="vp")
            nc.vector.memset(vp[:, :, 64], 1.0)
            nc.vector.memset(vp[:, :, 129], 1.0)

            # Projections are two-step: kick the x-chunk DMA one boundary
            # ahead, then run the accumulating matmuls + bias-add.
            xts = {}

            def proj_kick(xT, st):
                # prefetch via the idle Pool engine's SWDGE queue: these
                # never wait on anything, so they must not sit behind
                # producer-gated DMAs in the SP queue
                xTr = xT[:].rearrange("(j p) s -> p j s", p=128)
                xt = xin.tile([128, CJ, 512], BF16, tag="xin")
                nc.gpsimd.dma_start(
                    out=xt, in_=xTr[:, :, st * 512 : (st + 1) * 512]
                )
                xts[(id(xT), st)] = xt

            def proj_mm(xT, wname, dst, st):
                xt = xts.pop((id(xT), st))
                ps = mmps.tile([128, 512], F32, tag="mm")
                for cj in range(CJ):
                    nc.tensor.matmul(
                        ps,
                        lhsT=w_sb[wname][:, cj, :],
                        rhs=xt[:, cj, :],
                        start=(cj == 0),
                        stop=(cj == CJ - 1),
                    )
                nc.vector.tensor_scalar_add(
                    dst[:, st * 512 : (st + 1) * 512], ps, b_sb["b" + wname[1]]
                )

            def v_mm(st):
                """Project V s-tile, then xbar-DMA-transpose into vp.

                The xbar transpose needs a contiguous destination (strided
                out-APs produce garbage on hardware), so it lands in a
                scratch tile and a DVE copy scatters it into vp's
                ones-interleaved layout."""
                proj_mm(xvT, "wv", vpT, st)
                vtr = xin.tile([128, 2, 4, 64], BF16, tag="vtr", name="vtr")
                for h in (0, 1):
                    nc.sync.dma_start_transpose(
                        out=vtr[:, h, :, :],
                        in_=vpT[64 * h : 64 * h + 64, st * 512 : (st + 1) * 512],
                    )
                    nc.vector.tensor_copy(
                        vp[:, 4 * st : 4 * st + 4, 65 * h : 65 * h + 64],
                        vtr[:, h, :, :],
                    )

            # --- epilogue helpers ---
            state = {}

            def normalize(qt, h):
                cps = state[("cps", qt, h)]
                ctxQ = state[("ctxQ", qt)]
                rec = normpool.tile([128, 4], F32, tag="rec")
                nc.vector.reciprocal(rec, cps[:, :, 64])
                for qc in range(4):
                    nc.vector.tensor_scalar_mul(
                        ctxQ[:, qc, 64 * h : 64 * h + 64],
                        cps[:, qc, 0:64],
                        rec[:, qc : qc + 1],
                    )

            def trcopy(qt):
                """ctxT = transpose(ctxQ) via PE + copies (one PSUM bank)."""
                ctxQ = state[("ctxQ", qt)]
                ctxT = ctxpool.tile([DL, 512], BF16, tag="ctxT", name="ctxT")
                state[("ctxT", qt)] = ctxT
                mmt = mmps.tile([128, 512], F32, tag="mm")
                for qc in range(4):
                    nc.tensor.transpose(
                        mmt[:, qc * 64 : (qc + 1) * 64].bitcast(BF16),
                        ctxQ[:, qc, :],
                        ident,
                    )
                for qc in range(4):
                    nc.vector.tensor_copy(
                        ctxT[:, qc * 128 : (qc + 1) * 128],
                        mmt[:, qc * 64 : (qc + 1) * 64].bitcast(BF16),
                    )

            def oproj_j(qt, j):
                ops = mmps.tile([128, 512], F32, tag="mm")
                nc.tensor.matmul(
                    ops,
                    lhsT=wo_bf[:, j * 128 : (j + 1) * 128],
                    rhs=state[("ctxT", qt)],
                    start=True,
                    stop=True,
                )
                ot = outpool.tile([128, 512], BF16, tag="out")
                nc.vector.tensor_copy(ot, ops)
                nc.sync.dma_start(
                    out=outT[j * 128 : (j + 1) * 128, qt * 512 : (qt + 1) * 512], in_=ot
                )

            # --- boundary-job schedule ---
            pre = {}
            post = {}

            def add(d, key, fn):
                d.setdefault(key, []).append(fn)

            # qt0 pass h0: K s-tiles ahead of the scores needing them, V
            # s-tiles ahead of the (lagged) ctx matmuls, DMA kicks one
            # boundary earlier still.
            kick = lambda xT, st: (lambda: proj_kick(xT, st))
            for st, c in [(3, 2), (4, 4), (5, 6), (6, 7), (7, 8)]:
                add(pre, (0, 0, c), kick(xvT, st))
            for st, c in [(2, 0), (3, 1), (4, 2), (5, 3), (6, 6), (7, 7)]:
                add(pre, (0, 0, c), (lambda st=st: proj_mm(xkT, "wk", kpT, st)))
            for st, c in [(1, 4), (2, 5), (3, 8), (4, 9), (5, 10)]:
                add(pre, (0, 0, c), (lambda st=st: v_mm(st)))
            add(pre, (0, 1, 0), (lambda: v_mm(6)))
            add(pre, (0, 1, 1), (lambda: v_mm(7)))
            add(post, (0, 0, 5), wo_job)
            # normalize as soon as a pass's last ctx batch has popped;
            # transposes/output projection of qt during qt+1 pass h0; Q
            # projection of qt+1 during qt pass h1.
            for qt in range(QT):
                add(post, (qt, 1, CTX_LAG - 1), (lambda qt=qt: normalize(qt, 0)))
            for qt in range(1, QT):
                add(post, (qt, 0, CTX_LAG - 1), (lambda qt=qt: normalize(qt - 1, 1)))
                add(post, (qt, 0, CTX_LAG), (lambda qt=qt: trcopy(qt - 1)))
                for j in range(4):
                    add(post, (qt, 0, CTX_LAG + 1 + j),
                        (lambda qt=qt, j=j: oproj_j(qt - 1, j)))
            for qt in range(QT - 1):
                add(post, (qt, 1, CTX_LAG - 1), kick(xqT, qt + 1))
                add(post, (qt, 1, CTX_LAG + 1), (lambda qt=qt: proj_mm(
                    xqT, "wq", qpT, qt + 1)))

            load_w("wq", wq, bq)
            proj_kick(xqT, 0)
            load_w("wk", wk, bk)
            proj_kick(xkT, 0)
            proj_mm(xqT, "wq", qpT, 0)
            proj_kick(xkT, 1)
            load_w("wv", wv, bv)
            proj_kick(xvT, 0)
            proj_mm(xkT, "wk", kpT, 0)
            for _s in (2, 3, 4, 5, 6, 7):
                proj_kick(xkT, _s)
            proj_kick(xvT, 1)
            proj_kick(xvT, 2)
            proj_mm(xkT, "wk", kpT, 1)
            v_mm(0)

            # --- attention: head-sequential passes per q-tile ---
            def scores_chunk(sp, qt, h, k0, k1):
                qs = slice(qt * 512, (qt + 1) * 512)
                hs = slice(64 * h, 64 * h + 64)
                for i in range(k1 - k0):
                    kb = k0 + i
                    nc.tensor.matmul(
                        sp[:, i * 512 : (i + 1) * 512],
                        lhsT=kpT[hs, kb * 128 : (kb + 1) * 128],
                        rhs=qpT[hs, qs],
                        start=True,
                        stop=True,
                    )

            def ctx_batch(cps, h, pt, k0, k1):
                for i in range(k1 - k0):
                    kb = k0 + i
                    for qc in range(4):
                        nc.tensor.matmul(
                            cps[:, qc, :],
                            lhsT=pt[:, i * 512 + qc * 128 : i * 512 + (qc + 1) * 128],
                            rhs=vp[:, kb, 65 * h : 65 * h + 65],
                            start=False,
                            stop=(kb == KB - 1 and qc == 3),
                            skip_group_check=True,
                        )

            from collections import deque

            pending = deque()
            for qt in range(QT):
                state[("ctxQ", qt)] = ctxpool.tile(
                    [128, 4, DL], BF16, tag="ctxQ", name="ctxQ"
                )
                for h in (0, 1):
                    cps = cpsp.tile([128, 4, 65], F32, tag="cps")
                    state[("cps", qt, h)] = cps
                    # explicit zero of the accumulator region: hardware's
                    # lazy zero-on-start does not cover interleaved groups
                    nc.tensor.matmul(
                        cps[:].rearrange("p a b -> p (a b)"),
                        lhsT=ident,
                        rhs=zeros260,
                        start=True,
                        stop=False,
                        skip_group_check=True,
                    )
                    for c, (k0, k1) in enumerate(CHUNKS):
                        for fn in pre.get((qt, h, c), ()):
                            fn()
                        w = 512 * (k1 - k0)
                        sp = spring.tile([128, 1536], F32, tag="sp")
                        scores_chunk(sp, qt, h, k0, k1)
                        pt = ptpool.tile([128, 1536], BF16, tag="pt")
                        nc.scalar.activation(pt[:, 0:w], sp[:, 0:w], EXP, scale=SCALE)
                        if len(pending) >= CTX_LAG:
                            ctx_batch(*pending.popleft())
                        for fn in post.get((qt, h, c), ()):
                            fn()
                        pending.append((cps, h, pt, k0, k1))

            while pending:
                ctx_batch(*pending.popleft())
            normalize(QT - 1, 1)
            trcopy(QT - 1)
            for j in range(4):
                oproj_j(QT - 1, j)
    return nc


def _build(reps=1):
    if reps not in _prog_cache:
        nc = bacc.Bacc()
        _emit(nc, reps)
        nc.compile()
        _prog_cache[reps] = nc
    return _prog_cache[reps]


def _make_in_maps(query, key, value, Wq, bq, Wk, bk, Wv, bv, Wo):
    import ml_dtypes

    bf = ml_dtypes.bfloat16
    in_maps = []
    for b in range(B):
        xqT = np.ascontiguousarray(query[b].T.astype(bf))
        xkT = np.ascontiguousarray(key[b].T.astype(bf))
        xvT = np.ascontiguousarray(value[b].T.astype(bf))
        for hp in range(HP):
            cs = slice(hp * DL, (hp + 1) * DL)
            in_maps.append(
                {
                    "xqT": xqT,
                    "xkT": xkT,
                    "xvT": xvT,
                    "wq": np.ascontiguousarray(Wq[:, cs].astype(bf)),
                    "wk": np.ascontiguousarray(Wk[:, cs].astype(bf)),
                    "wv": np.ascontiguousarray(Wv[:, cs].astype(bf)),
                    "bq": np.ascontiguousarray(bq[cs].reshape(DL, 1)),
                    "bk": np.ascontiguousarray(bk[cs].reshape(DL, 1)),
                    "bv": np.ascontiguousarray(bv[cs].reshape(DL, 1)),
                    "wo": np.ascontiguousarray(Wo[cs, :].astype(bf)),
                }
            )
    return in_maps


def kernel(query, key, value, Wq, bq, Wk, bk, Wv, bv, Wo, bo):
    global LAST_RESULTS
    query = np.asarray(query, dtype=np.float32)
    key = np.asarray(key, dtype=np.float32)
    value = np.asarray(value, dtype=np.float32)
    Wq = np.asarray(Wq, dtype=np.float32)
    Wk = np.asarray(Wk, dtype=np.float32)
    Wv = np.asarray(Wv, dtype=np.float32)
    Wo = np.asarray(Wo, dtype=np.float32)
    bq = np.asarray(bq, dtype=np.float32)
    bk = np.asarray(bk, dtype=np.float32)
    bv = np.asarray(bv, dtype=np.float32)
    bo = np.asarray(bo, dtype=np.float32)

    nc = _build()
    in_maps = _make_in_maps(query, key, value, Wq, bq, Wk, bk, Wv, bv, Wo)

    res = run_bass_kernel_spmd(nc, in_maps, list(range(B * HP)), trace=TRACE)
    LAST_RESULTS = res

    out = np.empty((B, S, D), dtype=np.float32)
    for b in range(B):
        acc = res.results[b * HP]["outT"].astype(np.float32)
        for hp in range(1, HP):
            acc = acc + res.results[b * HP + hp]["outT"]
        out[b] = acc.T + bo
    return out


# revision 24
# speedup vs baseline: 1.0609x; 1.0609x over previous
"""MultiHeadedAttention Trainium2 kernel.

Problem: B=2, S=4096, d_model=512, H=8 heads, dk=64.
  q/k/v proj -> scaled dot-product attention per head -> concat -> out proj.

Sharding: 8 cores = (batch b in {0,1}) x (head-pair hp in {0..3}).
Each core computes, for its batch and its 2 heads:
  QpT/KpT/VpT = (x @ W[:, hp*128:hp*128+128] + b).T          [128=2*dk, 4096]
  S^T[k, q]   = Kp Qp^T (per head, fp32r matmuls, k-major)
  P^T         = exp(S^T / 8)  (ScalarE, reads PSUM, writes SBUF)
  ctxT, Z     = [Vp | 1].T @ P^T  (ones column gives softmax denominator)
  ctxT       /= Z  (reciprocal + partition_broadcast + multiply)
  outT_partial= Wo[hp*128:...]^T-chunks @ ctx   -> [512, 4096] (transposed)
Host: out[b] = sum_hp(outT_partial).T + bo.

Inputs are fed pre-transposed ([d_model, S], c-major) so projection matmuls
contract over the partition dimension with no on-device transposes of x.
"""

import numpy as np

import concourse.bass as bass
import concourse.bacc as bacc
import concourse.mybir as mybir
import concourse.tile as tile
from concourse.bass_utils import run_bass_kernel_spmd
from concourse.masks import make_identity

F32 = mybir.dt.float32
F32R = mybir.dt.float32r
EXP = mybir.ActivationFunctionType.Exp

B = 2
S = 4096
D = 512           # d_model
H = 8
DK = 64
HP = 4            # head pairs per batch
DL = 128          # local channels per core (2 heads)
CJ = 4            # contraction chunks of 128 over d_model
QT = S // 512     # 8 q-tiles of 512
KB = S // 128     # 32 k-blocks of 128
SCALE = 1.0 / np.sqrt(DK).item()  # 1/8
CTX_LAG = 6

TRACE = False
LAST_RESULTS = None

_prog_cache = {}


def _emit(nc, reps=1):
    xqT = nc.declare_dram_parameter("xqT", [D, S], F32, isOutput=False)
    xkT = nc.declare_dram_parameter("xkT", [D, S], F32, isOutput=False)
    xvT = nc.declare_dram_parameter("xvT", [D, S], F32, isOutput=False)
    wq = nc.declare_dram_parameter("wq", [D, DL], F32, isOutput=False)
    wk = nc.declare_dram_parameter("wk", [D, DL], F32, isOutput=False)
    wv = nc.declare_dram_parameter("wv", [D, DL], F32, isOutput=False)
    bq = nc.declare_dram_parameter("bq", [DL, 1], F32, isOutput=False)
    bk = nc.declare_dram_parameter("bk", [DL, 1], F32, isOutput=False)
    bv = nc.declare_dram_parameter("bv", [DL, 1], F32, isOutput=False)
    wo = nc.declare_dram_parameter("wo", [DL, D], F32, isOutput=False)
    outT = nc.declare_dram_parameter("outT", [D, S], F32, isOutput=True)

    with tile.TileContext(nc) as tc:
        with (
            nc.allow_low_precision(reason="fp32r (fp22-mantissa) matmul inputs"),
            tc.tile_pool(name="singles", bufs=1) as singles,
            tc.tile_pool(name="xin", bufs=4) as xin,
            tc.tile_pool(name="proj", bufs=1) as proj,
            tc.tile_pool(name="pt", bufs=10) as ptpool,
            tc.tile_pool(name="ctx", bufs=2) as ctxpool,
            tc.tile_pool(name="outp", bufs=4) as outpool,
            tc.tile_pool(name="norm", bufs=6) as normpool,
            tc.tile_pool(name="mm512", bufs=2, space="PSUM") as mmps,
            tc.tile_pool(name="sps", bufs=2, space="PSUM") as spsum,
            tc.tile_pool(name="cps", bufs=2, space="PSUM") as cpsum,
        ):
            # --- constants / weights ---
            ident = singles.tile([128, 128], F32, tag="ident")
            make_identity(nc, ident)
            ones_stage = singles.tile([128, 64], F32, tag="ones_stage")
            nc.vector.memset(ones_stage, 1.0)
            ones64 = singles.tile([1, 64], F32R, tag="ones64")
            nc.vector.tensor_copy(ones64, ones_stage[0:1, :])
            warm = singles.tile([1, 8], F32, tag="warm")
            nc.scalar.activation(warm, ones_stage[0:1, 0:8], EXP, scale=1.0)

            w_sb = {}
            b_sb = {}

            def load_w(name, w, bias):
                t = singles.tile([128, CJ, DL], F32R, tag=name)
                nc.sync.dma_start(
                    out=t,
                    in_=w[:].rearrange("(j p) d -> p j d", p=128).bitcast(F32R),
                )
                w_sb[name] = t
                bt = singles.tile([DL, 1], F32, tag="b" + name[1])
                nc.sync.dma_start(out=bt, in_=bias[:])
                b_sb["b" + name[1]] = bt

            # --- projections: dst = (x @ W + b).T, channel-major [128, S] ---
            qpT = proj.tile([DL, S], F32R, tag="qpT")
            kpT = proj.tile([DL, S], F32R, tag="kpT")
            vpT = proj.tile([DL, S], F32, tag="vpT")

            vp = proj.tile([128, KB, 130], F32R, tag="vp")
            nc.vector.tensor_copy(vp[:, :, 64], ones_stage[:, 0:KB])
            nc.vector.tensor_copy(vp[:, :, 129], ones_stage[:, 0:KB])

            def project_cols(xT, wname, dst, c0, w):
                """Columns [c0, c0+w) of dst = (x @ W + b).T"""
                xTr = xT[:].rearrange("(j p) s -> p j s", p=128)
                xt = xin.tile([128, CJ, 512], F32R, tag="xin")
                nc.sync.dma_start(
                    out=xt[:, :, 0:w],
                    in_=xTr[:, :, c0 : c0 + w].bitcast(F32R),
                )
                ps = mmps.tile([128, 512], F32, tag="mm512")
                for cj in range(CJ):
                    nc.tensor.matmul(
                        ps[:, 0:w],
                        lhsT=w_sb[wname][:, cj, :],
                        rhs=xt[:, cj, 0:w],
                        start=(cj == 0),
                        stop=(cj == CJ - 1),
                    )
                nc.vector.tensor_scalar_add(
                    dst[:, c0 : c0 + w], ps[:, 0:w], b_sb["b" + wname[1]]
                )

            def project_st(xT, wname, dst, st):
                project_cols(xT, wname, dst, st * 512, 512)

            def v_transpose_st(st):
                """Vp s-major blocks for the 4 k-blocks of one s-tile."""
                for kb in range(st * 4, st * 4 + 4):
                    tp = mmps.tile([128, 512], F32, tag="mm512")
                    nc.tensor.transpose(
                        tp[:, 0:128], vpT[:, kb * 128 : (kb + 1) * 128], ident
                    )
                    nc.vector.tensor_copy(vp[:, kb, 0:64], tp[:, 0:64])
                    nc.vector.tensor_copy(vp[:, kb, 65:129], tp[:, 64:128])

            # Streaming order chosen so the attention frontier unlocks ASAP:
            # q-tile 0 first, then K/V interleaved per s-tile (each s-tile
            # unlocks 4 k-blocks for scores+ctx), remaining Q tiles last.
            load_w("wq", wq, bq)
            project_st(xqT, "wq", qpT, 0)
            load_w("wk", wk, bk)
            load_w("wv", wv, bv)
            wo_sb = singles.tile([DL, D], F32R, tag="wo")
            nc.sync.dma_start(out=wo_sb, in_=wo[:].bitcast(F32R))
            for st in range(QT):
                project_st(xkT, "wk", kpT, st)
                project_st(xvT, "wv", vpT, st)
                v_transpose_st(st)
            for st in range(1, QT):
                project_st(xqT, "wq", qpT, st)

            # --- attention + output projection, per q-tile of 512 ---
            # Epilogue work (normalize + Wo projection) for q-tile qt is
            # emitted piecewise during q-tile qt+1's kb loop so the PE queue
            # never stalls ACT at the boundary.
            state = {}

            def normalize_h(qt, cps_h, h):
                if h == 0:
                    state["ctx"] = ctxpool.tile([DL, 512], F32R, tag="ctx", name="ctx")
                rec = normpool.tile([1, 512], F32R, tag="rec")
                nc.vector.reciprocal(rec, cps_h[h][64:65, :])
                bc = normpool.tile([64, 512], F32, tag="bc")
                nc.gpsimd.partition_broadcast(bc, rec.bitcast(F32))
                nc.vector.tensor_mul(
                    state["ctx"][h * 64 : (h + 1) * 64, :], cps_h[h][0:64, :], bc
                )

            def oproj_j(qt, j):
                qs = slice(qt * 512, (qt + 1) * 512)
                ops = mmps.tile([128, 512], F32, tag="mm512")
                nc.tensor.matmul(
                    ops,
                    lhsT=wo_sb[:, j * 128 : (j + 1) * 128],
                    rhs=state["ctx"],
                    start=True,
                    stop=True,
                )
                ot = outpool.tile([128, 512], F32, tag="out")
                nc.vector.tensor_copy(ot, ops)
                nc.sync.dma_start(out=outT[j * 128 : (j + 1) * 128, qs], in_=ot)

            def epilogue_step(step, qt, cps_h):
                if step == 1:
                    normalize_h(qt, cps_h, 0)
                elif step == 2:
                    normalize_h(qt, cps_h, 1)
                elif 3 <= step <= 6:
                    oproj_j(qt, step - 3)

            def scores_exp(qt, kb):
                qs = slice(qt * 512, (qt + 1) * 512)
                sp = spsum.tile([128, 1024], F32, tag="sps")
                for h in (0, 1):
                    nc.tensor.matmul(
                        sp[:, h * 512 : (h + 1) * 512],
                        lhsT=kpT[h * 64 : (h + 1) * 64, kb * 128 : (kb + 1) * 128],
                        rhs=qpT[h * 64 : (h + 1) * 64, qs],
                        start=True,
                        stop=True,
                    )
                pt = ptpool.tile([128, 1024], F32R, tag="pt")
                nc.scalar.activation(pt, sp, EXP, scale=SCALE)
                return pt

            def ctx_mm(cps_h, kb, pt):
                for h in (0, 1):
                    nc.tensor.matmul(
                        cps_h[h][0:65, :],
                        lhsT=vp[:, kb, 65 * h : 65 * h + 65],
                        rhs=pt[:, h * 512 : (h + 1) * 512],
                        start=(kb == 0),
                        stop=(kb == KB - 1),
                    )

            # Software pipeline: ctx(kb-1) is emitted after scores/exp(kb) so
            # the in-order PE queue never makes ACT wait a full ctx+scores hop.
            pending = None  # (qt, cps_h) awaiting epilogue
            for qt in [q for _ in range(reps) for q in range(QT)]:
                cps0 = cpsum.tile([128, 512], F32, tag="cps")
                cps1 = cpsum.tile([128, 512], F32, tag="cps")
                cps_h = (cps0, cps1)
                pts = {}
                for kb in range(KB):
                    pts[kb] = scores_exp(qt, kb)
                    if kb >= CTX_LAG:
                        ctx_mm(cps_h, kb - CTX_LAG, pts.pop(kb - CTX_LAG))
                    if pending is not None:
                        epilogue_step(kb, *pending)
                for t in range(KB - CTX_LAG, KB):
                    ctx_mm(cps_h, t, pts.pop(t))
                pending = (qt, cps_h)
            for step in range(1, 7):
                epilogue_step(step, *pending)
    return nc


def _build(reps=1):
    if reps not in _prog_cache:
        nc = bacc.Bacc()
        _emit(nc, reps)
        nc.compile()
        _prog_cache[reps] = nc
    return _prog_cache[reps]


def _make_in_maps(query, key, value, Wq, bq, Wk, bk, Wv, bv, Wo):
    in_maps = []
    for b in range(B):
        xqT = np.ascontiguousarray(query[b].T)
        xkT = np.ascontiguousarray(key[b].T)
        xvT = np.ascontiguousarray(value[b].T)
        for hp in range(HP):
            cs = slice(hp * DL, (hp + 1) * DL)
            in_maps.append(
                {
                    "xqT": xqT,
                    "xkT": xkT,
                    "xvT": xvT,
                    "wq": np.ascontiguousarray(Wq[:, cs]),
                    "wk": np.ascontiguousarray(Wk[:, cs]),
                    "wv": np.ascontiguousarray(Wv[:, cs]),
                    "bq": np.ascontiguousarray(bq[cs].reshape(DL, 1)),
                    "bk": np.ascontiguousarray(bk[cs].reshape(DL, 1)),
                    "bv": np.ascontiguousarray(bv[cs].reshape(DL, 1)),
                    "wo": np.ascontiguousarray(Wo[cs, :]),
                }
            )
    return in_maps


def kernel(query, key, value, Wq, bq, Wk, bk, Wv, bv, Wo, bo):
    global LAST_RESULTS
    query = np.asarray(query, dtype=np.float32)
    key = np.asarray(key, dtype=np.float32)
    value = np.asarray(value, dtype=np.float32)
    Wq = np.asarray(Wq, dtype=np.float32)
    Wk = np.asarray(Wk, dtype=np.float32)
    Wv = np.asarray(Wv, dtype=np.float32)
    Wo = np.asarray(Wo, dtype=np.float32)
    bq = np.asarray(bq, dtype=np.float32)
    bk = np.asarray(bk, dtype=np.float32)
    bv = np.asarray(bv, dtype=np.float32)
    bo = np.asarray(bo, dtype=np.float32)

    nc = _build()
    in_maps = _make_in_maps(query, key, value, Wq, bq, Wk, bk, Wv, bv, Wo)

    res = run_bass_kernel_spmd(nc, in_maps, list(range(B * HP)), trace=TRACE)
    LAST_RESULTS = res

    out = np.empty((B, S, D), dtype=np.float32)
    for b in range(B):
        acc = res.results[b * HP]["outT"].astype(np.float32)
        for hp in range(1, HP):
            acc = acc + res.results[b * HP + hp]["outT"]
        out[b] = acc.T + bo
    return out

